# revision 9
# baseline (speedup 1.0000x reference)
"""Trainium2 Bass kernel for nn_CostVolume (SpatialCorrelationSampler-style).

out[b, dy*9+dx, y, x] = sum_c feat1[b,c,y,x] * feat2_pad[b,c,y+dy,x+dx]
with feat2 zero-padded by 4 on H/W, dy/dx in [0,9), B=4, C=256, H=W=96.

Sharding (8 cores): core = (b, half) — batch x H-half. Per core:
  f1  [256, 48, 96]    feat1[b, :, half*48:(half+1)*48, :]
  f2  [256, 56, 104]   pad(feat2[b])[:, half*48 : half*48+56, :]
  out O'[6, 3, 96, 8, 27] intermediate (deskewed on host)

Device algorithm per (y, g) (g = dy-group of 3):
  Gram G[x, n] = sum_c f1[c,y,x] * f2[c, y+3g+dyl, x'],  n = x'*3 + dyl
  computed as 2 accumulating f32r matmuls (C split 128+128) -> PSUM [96, 312].
  Useful band for partition x = 27 contiguous cols starting at 3x
  (j = 3*dx + dyl). PSUM -> SBUF staging (DVE/ACT), then one diagonal-AP
  DMA per (8-row block, g): src ap [[7488+3, 96], [936, 8], [1, 27]].
"""

import numpy as np
import ml_dtypes

import concourse.bacc as bacc
import concourse.mybir as mybir
from concourse.ap import AP
from concourse.tile import TileContext
from concourse.bass_utils import run_bass_kernel_spmd

B, C, H, W = 4, 256, 96, 96
D = 4            # max displacement; pad width
P = 2 * D + 1    # 9 displacements per axis
HH = H // 2      # 48 rows per core
IB = 16          # input y-block
SB = 8           # staging y-block
NIB = HH // IB   # 3 input blocks
NSB = IB // SB   # 2 staging sub-blocks per input block
WROW = 936 * SB  # staging row length (elements per partition)

F32 = mybir.dt.float32
F32R = mybir.dt.float32r
BF16 = mybir.dt.bfloat16
USE_BF16 = True
IN_DT = BF16 if USE_BF16 else F32R
STA_W = W   # stationary cols (FWL unavailable: ldw-opt incompatible)
MP = 96     # matmul output partitions

_CACHED = {}


def _build_nc():
    nc = bacc.Bacc()
    f1 = nc.declare_dram_parameter(
        "f1", [C, HH * W + (STA_W - W)], IN_DT, isOutput=False
    )
    f2 = nc.declare_dram_parameter("f2", [C, HH + 8, W + 8], IN_DT, isOutput=False)
    out = nc.declare_dram_parameter(
        "o", [HH // SB, 3, W, SB, 27], F32, isOutput=True
    )

    with TileContext(nc) as tc:
        with (
            tc.tile_pool(name="inp", bufs=2) as inp,
            tc.tile_pool(name="stage", bufs=3) as stp,
            tc.tile_pool(name="ps", bufs=2, space="PSUM") as psp,
        ):
            for blk in range(NIB):
                f1t = []
                f2t = []
                for ch in range(2):
                    t1 = inp.tile([128, IB * W + (STA_W - W)], IN_DT, tag=f"f1c{ch}")
                    nc.scalar.dma_start(
                        out=t1[:, :],
                        in_=f1[
                            ch * 128 : (ch + 1) * 128,
                            blk * IB * W : (blk + 1) * IB * W + (STA_W - W),
                        ],
                    )
                    f1t.append(t1)
                    t2 = inp.tile([128, IB + 8, W + 8], IN_DT, tag=f"f2c{ch}")
                    nc.scalar.dma_start(
                        out=t2[:, :, :],
                        in_=f2[
                            ch * 128 : (ch + 1) * 128,
                            blk * IB : blk * IB + IB + 8,
                            :,
                        ],
                    )
                    f2t.append(t2)

                for sub in range(NSB):
                    st = stp.tile([96, WROW], F32, tag="st")
                    for yl in range(SB):
                        yi = sub * SB + yl  # y within input block
                        # one 3-bank PSUM tile per y; matmul g at col g*512
                        ps = psp.tile([MP, 1536], F32, tag="ps")
                        for ch in range(2):
                            sta = f1t[ch][:, yi * W : yi * W + STA_W]
                            for g in range(3):
                                mov = f2t[ch][
                                    :, yi + 3 * g : yi + 3 * g + 3, :
                                ].rearrange("c r x -> c (r x)")
                                nc.tensor.matmul(
                                    ps[:, g * 512 : g * 512 + 312],
                                    lhsT=sta,
                                    rhs=mov,
                                    start=(ch == 0),
                                    stop=(ch == 1),
                                )
                        # single strided copy per y: PSUM (g, dyl, x') ->
                        # staging interleaved col = g*312 + x'*3 + dyl
                        psap = ps[0:96, :]
                        src = AP(
                            tensor=psap.tensor,
                            offset=psap.offset,
                            ap=[[1536, 96], [512, 3], [104, 3], [1, 104]],
                        )
                        stap0 = st[:, :]
                        dst = AP(
                            tensor=stap0.tensor,
                            offset=stap0.offset + yl * 936,
                            ap=[[WROW, 96], [312, 3], [1, 3], [3, 104]],
                        )
                        if yl % 2 == 0:
                            nc.vector.tensor_copy(dst, src)
                        else:
                            nc.scalar.copy(out=dst, in_=src)
                    # band extraction: one diagonal-AP DMA per g, 27-elem runs
                    stap = st[:, :]
                    for g in range(3):
                        src = AP(
                            tensor=stap.tensor,
                            offset=stap.offset + g * 312,
                            ap=[[WROW + 3, 96], [936, SB], [1, 27]],
                        )
                        eng = nc.sync if (sub * 3 + g) % 2 == 0 else nc.scalar
                        eng.dma_start(out=out[blk * NSB + sub, g], in_=src)
    nc.finalize()
    return nc


def kernel(feat1: np.ndarray, feat2: np.ndarray) -> np.ndarray:
    feat1 = np.ascontiguousarray(np.asarray(feat1, dtype=np.float32))
    feat2 = np.ascontiguousarray(np.asarray(feat2, dtype=np.float32))

    if "nc" not in _CACHED:
        _CACHED["nc"] = _build_nc()
    nc = _CACHED["nc"]

    core_ids = list(range(8))
    in_maps = []
    for core in core_ids:
        b, half = divmod(core, 2)
        f1s = feat1[b][:, half * HH : (half + 1) * HH, :].reshape(C, HH * W)
        f1s = np.concatenate(
            [f1s, np.zeros((C, STA_W - W), np.float32)], axis=1
        )
        f2p = np.pad(feat2[b], ((0, 0), (D, D), (D, D)))
        f2s = f2p[:, half * HH : half * HH + HH + 8, :]
        npdt = ml_dtypes.bfloat16 if USE_BF16 else np.float32
        in_maps.append(
            {
                "f1": np.ascontiguousarray(f1s.astype(npdt)),
                "f2": np.ascontiguousarray(f2s.astype(npdt)),
            }
        )

    res = run_bass_kernel_spmd(nc, in_maps, core_ids)

    out = np.empty((B, P * P, H, W), np.float32)
    for core in core_ids:
        b, half = divmod(core, 2)
        Op = res.results[core]["o"]  # [cb, g, x, yl, 27] with j = 3*dx + dyl
        O = Op.reshape(HH // SB, 3, W, SB, P, 3)  # cb, g, x, yl, dx, dyl
        core_out = O.transpose(1, 5, 4, 0, 3, 2).reshape(P * P, HH, W)
        out[b, :, half * HH : (half + 1) * HH, :] = core_out
    return out
